# revision 57
# baseline (speedup 1.0000x reference)
"""BatchGRU TRN2 kernel v3: fp8 DoubleRow x-side + bf16 h-side.

Host ships x pre-transposed ([300, NLOC] per core) and weights pre-packed:
x-side weights fp8e4m3 scaled x4096 in DoubleRow pair layout ([128,2,900] +
[32,2,900] tail with the xn-bias ones-row), h-side weights bf16 scaled x4096
([128,2,900] + [45,900] tail with the r/z/n bias row). Gate pre-activations
accumulate in PSUM at scale 4096 (one [128,3,512] tile per unit = r|z|xn|hn
bank-packed); sigmoid/tanh apply scale=1/4096. Recurrent state is bf16 in
k-chunk layout ([128,2,256] + [64,256] with ones row); h_new stays f32r in
natural layout (y DMA + h_old), PE-transposed and cast to bf16 on the
PSUM->SBUF copy. Elementwise work is spread ACT/DVE/Pool.
"""
import numpy as np
from contextlib import ExitStack

try:
    import jax as _jax
    _jax.config.update("jax_compilation_cache_dir", "/root/problem/jax_cache")
    _jax.config.update("jax_persistent_cache_min_compile_time_secs", 10.0)
    _jax.config.update("jax_persistent_cache_min_entry_size_bytes", 0)
except Exception:
    pass

import ml_dtypes
import concourse.bacc as bacc
import concourse.tile as tile
from concourse import mybir
from concourse.bass_utils import run_bass_kernel_spmd

f32 = mybir.dt.float32
f32r = mybir.dt.float32r
bf16 = mybir.dt.bfloat16
f8 = mybir.dt.float8e4
AF = mybir.ActivationFunctionType
ALU = mybir.AluOpType
DR = mybir.MatmulPerfMode.DoubleRow

NPF8 = ml_dtypes.float8_e4m3fn
NPBF = ml_dtypes.bfloat16

H = 300
G = 3 * H
L = 48
B = 2048
NCORES = 8
BLOC = B // NCORES           # 256 molecules per core
NLOC = BLOC * L              # 12288 tokens per core
NMC = 2                      # molecule chunks of 128
S = 4096.0                   # preact scale carried in the weights
INV_S = 1.0 / S
XONE = 128.0                 # ones-row value in fp8 x tiles
NCHUNK = 8                   # prologue token-column chunks
CW = NLOC // NCHUNK          # 3072 token cols per chunk
CM = BLOC // NCHUNK          # 64 molecules per chunk

# flat PSUM gate layout: r 0:300 | z 300:600 | xn 600:900 | hn 900:1200
# split at PSUM bank boundaries (512) and <=256-col pieces for DoubleRow.
# (flat0, flat1, has_x, has_h); weight col = flat (x) / flat-300*... see emit.
SUBREGIONS = [
    (0, 256, True, True),
    (256, 300, True, True),
    (300, 512, True, True),
    (512, 600, True, True),
    (600, 856, True, False),
    (856, 900, True, False),
    (900, 1024, False, True),
    (1024, 1200, False, True),
]

_cached = {}


def build_program():
    if "nc" in _cached:
        return _cached["nc"]
    nc = bacc.Bacc("TRN2", target_bir_lowering=False, debug=False,
                   dynamic_dma_scratch_size=512)

    xT_d = nc.declare_dram_parameter("xT", [H, NLOC], bf16, isOutput=False)
    wx_m_d = {d: nc.declare_dram_parameter(f"wx_m_{d}", [128, 2, G], f8,
                                           isOutput=False) for d in "fb"}
    wx_t_d = {d: nc.declare_dram_parameter(f"wx_t_{d}", [32, 2, G], f8,
                                           isOutput=False) for d in "fb"}
    wh_a_d = {d: nc.declare_dram_parameter(f"wh_a_{d}", [128, 2, G], bf16,
                                           isOutput=False) for d in "fb"}
    wh_b_d = {d: nc.declare_dram_parameter(f"wh_b_{d}", [45, G], bf16,
                                           isOutput=False) for d in "fb"}
    brelu_m_d = nc.declare_dram_parameter("brelu_m", [128, 2], f32, isOutput=False)
    brelu_t_d = nc.declare_dram_parameter("brelu_t", [22, 2], f32, isOutput=False)
    y_d = nc.declare_dram_parameter("y", [NLOC, 2 * H], f32, isOutput=True)

    y_r = y_d[:].rearrange("(m l) c -> m l c", l=L)  # [256, 48, 600]
    # p-major view for single-DMA stores of the merged [128, 2, H] hnat tile
    y_p = y_d[:].rearrange("(mc p l) c -> p mc l c", mc=NMC, l=L)

    with tile.TileContext(nc) as tc:
        with ExitStack() as ctx:
            consts = ctx.enter_context(tc.tile_pool(name="consts", bufs=1))

            # fp8 relu(x+bias), DoubleRow pair layout; tail holds ones row at
            # partition 22 (j=0) for the xn bias.
            xq_m = consts.tile([128, 2, NLOC], f8, name="xq_m")
            xq_t = consts.tile([32, 2, NLOC], f8, name="xq_t")
            wx_m = {d: consts.tile([128, 2, G], f8, name=f"wx_m_{d}") for d in "fb"}
            wx_t = {d: consts.tile([32, 2, G], f8, name=f"wx_t_{d}") for d in "fb"}
            wh_a = {d: consts.tile([128, 2, G], bf16, name=f"wh_a_{d}") for d in "fb"}
            wh_b = {d: consts.tile([45, G], bf16, name=f"wh_b_{d}") for d in "fb"}
            brelu_m = consts.tile([128, 2], f32)
            brelu_t = consts.tile([22, 2], f32)
            # bf16 state, k-chunk layout: a: hdim 128j+p; b: hdim 256+p (p<44),
            # ones row at p=44 (memset covers 32:64).
            state_a = {d: consts.tile([128, 2, BLOC], bf16, name=f"sa_{d}")
                       for d in "fb"}
            state_b = {d: consts.tile([64, BLOC], bf16, name=f"sb_{d}")
                       for d in "fb"}
            hnat = {d: consts.tile([128, NMC, H], f32r, name=f"hnat_{d}")
                    for d in "fb"}
            id128r = consts.tile([128, 128], f32r)
            id_bf = consts.tile([128, 128], bf16)

            # ---- identities ----
            with tc.tile_pool(name="idp", bufs=1) as idp:
                rowi = idp.tile([128, 1], mybir.dt.int32)
                coli = idp.tile([128, 128], mybir.dt.int32)
                nc.gpsimd.iota(rowi, pattern=[[0, 1]], base=0, channel_multiplier=1)
                nc.gpsimd.iota(coli, pattern=[[1, 128]], base=0, channel_multiplier=0)
                rowf = idp.tile([128, 1], f32)
                colf = idp.tile([128, 128], f32)
                nc.vector.tensor_copy(out=rowf, in_=rowi)
                nc.vector.tensor_copy(out=colf, in_=coli)
                idf = idp.tile([128, 128], f32)
                nc.vector.tensor_scalar(out=idf, in0=colf, scalar1=rowf,
                                        scalar2=None, op0=ALU.is_equal)
                nc.scalar.activation(out=id128r, in_=idf, func=AF.Copy)
                nc.scalar.activation(out=id_bf, in_=idf, func=AF.Copy)

            # ---- weights + biases ----
            for d in "fb":
                nc.sync.dma_start(out=wx_m[d], in_=wx_m_d[d][:])
                nc.sync.dma_start(out=wx_t[d], in_=wx_t_d[d][:])
                nc.sync.dma_start(out=wh_a[d], in_=wh_a_d[d][:])
                nc.sync.dma_start(out=wh_b[d], in_=wh_b_d[d][:])
            nc.sync.dma_start(out=brelu_m, in_=brelu_m_d[:])
            nc.sync.dma_start(out=brelu_t, in_=brelu_t_d[:])

            # ones rows (overwritten on 0:22 by the relu writers below);
            # on Pool: these are 12288-wide and DVE is the prologue bottleneck
            nc.gpsimd.memset(xq_t[0:32, 0, :], XONE)
            nc.gpsimd.memset(xq_t[0:32, 1, :], 0.0)
            nc.gpsimd.memset(state_b["f"][32:64, :], 1.0)

            # ---- prologue: chunked x load, relu->fp8, h0 reduce ----
            with tc.tile_pool(name="xstage", bufs=2) as xstage:
                for c in range(NCHUNK):
                    cs = slice(c * CW, (c + 1) * CW)
                    ms = slice(c * CM, (c + 1) * CM)
                    st = xstage.tile([128, 2, CW], bf16, tag="stm")
                    stt = xstage.tile([22, 2, CW], bf16, tag="stt")
                    nc.sync.dma_start(out=st[:, 0, :], in_=xT_d[0:128, cs])
                    nc.sync.dma_start(out=st[:, 1, :], in_=xT_d[128:256, cs])
                    nc.sync.dma_start(out=stt[:, 0, :], in_=xT_d[256:278, cs])
                    nc.sync.dma_start(out=stt[:, 1, :], in_=xT_d[278:300, cs])
                    for j in range(2):
                        nc.scalar.activation(out=xq_m[:, j, cs], in_=st[:, j, :],
                                             func=AF.Relu,
                                             bias=brelu_m[:, j:j + 1], scale=1.0)
                        nc.scalar.activation(out=xq_t[0:22, j, cs], in_=stt[:, j, :],
                                             func=AF.Relu,
                                             bias=brelu_t[:, j:j + 1], scale=1.0)
                    # h0 = per-molecule max over raw x
                    st_v = st.rearrange("p j (m l) -> p j m l", l=L)
                    stt_v = stt.rearrange("p j (m l) -> p j m l", l=L)
                    for j in range(2):
                        nc.vector.tensor_reduce(out=state_a["f"][:, j, ms],
                                                in_=st_v[:, j],
                                                axis=mybir.AxisListType.X, op=ALU.max)
                    nc.vector.tensor_reduce(out=state_b["f"][0:22, ms],
                                            in_=stt_v[:, 0], axis=mybir.AxisListType.X,
                                            op=ALU.max)
                    # engine writes must start at a 32-aligned partition, so
                    # land the j=1 tail h0 at base 0 and DMA-shift it to 22:44
                    stgb = xstage.tile([22, CM], bf16, tag="stgb")  # noqa
                    nc.vector.tensor_reduce(out=stgb, in_=stt_v[:, 1],
                                            axis=mybir.AxisListType.X, op=ALU.max)
                    nc.sync.dma_start(out=state_b["f"][22:44, ms], in_=stgb)

            # state copy f -> b (incl. ones rows); ACT: DVE owns the reduces
            nc.scalar.activation(out=state_a["b"], in_=state_a["f"],
                                 func=AF.Copy)
            nc.scalar.activation(out=state_b["b"], in_=state_b["f"],
                                 func=AF.Copy)

            # h0 natural layout via PE transposes of the bf16 state
            with tc.tile_pool(name="h0t", bufs=2, space="PSUM") as h0t:
                for mc in range(NMC):
                    msl = slice(mc * 128, (mc + 1) * 128)
                    tp = h0t.tile([128, H], bf16, tag="h0")
                    nc.tensor.transpose(out=tp[:, 0:128],
                                        in_=state_a["f"][:, 0, msl], identity=id_bf)
                    nc.tensor.transpose(out=tp[:, 128:256],
                                        in_=state_a["f"][:, 1, msl], identity=id_bf)
                    nc.tensor.transpose(out=tp[:, 256:300],
                                        in_=state_b["f"][0:44, msl],
                                        identity=id_bf[0:44, 0:44])
                    nc.scalar.activation(out=hnat["f"][:, mc, :], in_=tp,
                                         func=AF.Copy)
                    nc.gpsimd.tensor_copy(out=hnat["b"][:, mc, :],
                                          in_=hnat["f"][:, mc, :].bitcast(f32))

            xqm_v = xq_m.rearrange("p j (m l) -> p j m l", l=L)
            xqt_v = xq_t.rearrange("p j (m l) -> p j m l", l=L)

            # ---- recurrence pools ----
            gp_pool = ctx.enter_context(tc.tile_pool(name="gp", bufs=2, space="PSUM"))
            tp_pool = ctx.enter_context(tc.tile_pool(name="tp", bufs=2, space="PSUM"))
            gates = ctx.enter_context(tc.tile_pool(name="gates", bufs=2))

            def emit_mm(d, s, mc):
                t = s if d == "f" else L - 1 - s
                sa, sb = state_a[d], state_b[d]
                msl = slice(mc * 128, (mc + 1) * 128)
                g_ps = gp_pool.tile([128, 3, 512], f32, tag="g",
                                    name=f"g_{d}{s}{mc}")
                gf = g_ps.rearrange("p a b -> p (a b)")
                for (f0, f1, has_x, has_h) in SUBREGIONS:
                    first = True
                    if has_x:
                        wsl = slice(f0, f1)
                        nc.tensor.matmul(
                            out=gf[:, f0:f1], lhsT=xqm_v[:, :, msl, t],
                            rhs=wx_m[d][:, :, wsl],
                            start=True, stop=False, perf_mode=DR)
                        nc.tensor.matmul(
                            out=gf[:, f0:f1], lhsT=xqt_v[0:23, :, msl, t],
                            rhs=wx_t[d][0:23, :, wsl],
                            start=False, stop=not has_h, perf_mode=DR)
                        first = False
                    if has_h:
                        w0 = f0 if f0 < 900 else f0 - 300
                        wsl = slice(w0, w0 + (f1 - f0))
                        for k in range(2):
                            nc.tensor.matmul(
                                out=gf[:, f0:f1], lhsT=sa[:, k, msl],
                                rhs=wh_a[d][:, k, wsl],
                                start=first and k == 0, stop=False)
                        nc.tensor.matmul(
                            out=gf[:, f0:f1], lhsT=sb[0:45, msl],
                            rhs=wh_b[d][0:45, wsl], start=False, stop=True)
                return gf

            def emit_gm(d, s, mc, gf):
                # gate math chain only; transposes/copies are emitted
                # separately (deferred for cross-step pipelining)
                t = s if d == "f" else L - 1 - s
                dcol = 0 if d == "f" else 1
                hn_ = hnat[d][:, mc, :]
                msl = slice(mc * 128, (mc + 1) * 128)
                rz = gates.tile([128, 2, H], f32, tag="rz", name=f"rz_{d}{s}{mc}")
                t1 = gates.tile([128, H], f32, tag="t1", name=f"t1_{d}{s}{mc}")
                n_sb = gates.tile([128, H], f32, tag="ns", name=f"ns_{d}{s}{mc}")
                nc.scalar.activation(out=rz, in_=gf[:, 0:600],
                                     func=AF.Sigmoid, scale=INV_S)
                nc.vector.tensor_mul(out=t1, in0=rz[:, 0, :], in1=gf[:, 900:1200])
                nc.vector.tensor_add(out=t1, in0=t1, in1=gf[:, 600:900])
                nc.scalar.activation(out=n_sb, in_=t1, func=AF.Tanh, scale=INV_S)
                nc.gpsimd.tensor_sub(out=t1, in0=hn_.bitcast(f32), in1=n_sb)
                nc.vector.tensor_mul(out=t1, in0=rz[:, 1, :], in1=t1)
                nc.gpsimd.tensor_add(out=hn_, in0=n_sb, in1=t1)
                if mc == NMC - 1:
                    # one DMA per (d, step): both mc chunks from the merged
                    # hnat tile; DRAM AP iterates (p, mc, c) to match
                    nc.sync.dma_start(
                        out=y_p[:, :, t, dcol * H:(dcol + 1) * H],
                        in_=hnat[d].bitcast(f32))

            def emit_tp(d, s, mc):
                hn_ = hnat[d][:, mc, :]
                tp = tp_pool.tile([128, 384], f32r, tag="tp", name=f"tp_{d}{s}{mc}")
                nc.tensor.transpose(out=tp[:, 0:128], in_=hn_[:, 0:128],
                                    identity=id128r)
                nc.tensor.transpose(out=tp[:, 128:256], in_=hn_[:, 128:256],
                                    identity=id128r)
                nc.tensor.transpose(out=tp[0:44, 256:384], in_=hn_[:, 256:300],
                                    identity=id128r)
                return tp

            def emit_copies(d, mc, tp):
                sa, sb = state_a[d], state_b[d]
                msl = slice(mc * 128, (mc + 1) * 128)
                # split across ACT/DVE: the copies gate the next step's
                # h-side matmuls, so parallelize rather than batch
                nc.scalar.activation(out=sa[:, 0, msl],
                                     in_=tp[:, 0:128].bitcast(f32), func=AF.Copy)
                nc.vector.tensor_copy(out=sa[:, 1, msl],
                                      in_=tp[:, 128:256].bitcast(f32))
                nc.vector.tensor_copy(out=sb[0:44, msl],
                                      in_=tp[0:44, 256:384].bitcast(f32))

            for s in range(L):
                for d in "fb":
                    gfs = [emit_mm(d, s, mc) for mc in range(NMC)]
                    for mc in range(NMC):
                        emit_gm(d, s, mc, gfs[mc])
                        if s < L - 1:  # final state is never read back
                            tp = emit_tp(d, s, mc)
                            emit_copies(d, mc, tp)

    nc.compile()
    _cached["nc"] = nc
    return nc


def _prep_shared_inputs(bias, w_ih_f, w_hh_f, b_ih_f, b_hh_f,
                        w_ih_b, w_hh_b, b_ih_b, b_hh_b):
    def pack_dir(w_ih, w_hh, b_ih, b_hh):
        Wx = (np.asarray(w_ih, np.float64).T * S).astype(np.float32)  # [300, 900]
        Wh = (np.asarray(w_hh, np.float64).T * S).astype(np.float32)
        wx_m = np.zeros((128, 2, G), np.float32)
        wx_t = np.zeros((32, 2, G), np.float32)
        for j in range(2):
            wx_m[:, j, :] = Wx[128 * j:128 * (j + 1), :]
            wx_t[0:22, j, :] = Wx[256 + 22 * j:256 + 22 * (j + 1), :]
        # xn bias rides the fp8 ones row (value XONE) at partition 22, j=0
        wx_t[22, 0, 2 * H:] = np.asarray(b_ih, np.float64)[2 * H:] * (S / XONE)
        wh_a = np.zeros((128, 2, G), np.float32)
        for j in range(2):
            wh_a[:, j, :] = Wh[128 * j:128 * (j + 1), :]
        wh_b = np.zeros((45, G), np.float32)
        wh_b[0:44, :] = Wh[256:300, :]
        brow = np.asarray(b_hh, np.float64).copy()
        brow[:2 * H] += np.asarray(b_ih, np.float64)[:2 * H]
        wh_b[44, :] = (brow * S).astype(np.float32)
        return (wx_m.astype(NPF8), wx_t.astype(NPF8),
                wh_a.astype(NPBF), wh_b.astype(NPBF))

    wx_m_f, wx_t_f, wh_a_f, wh_b_f = pack_dir(w_ih_f, w_hh_f, b_ih_f, b_hh_f)
    wx_m_b, wx_t_b, wh_a_b, wh_b_b = pack_dir(w_ih_b, w_hh_b, b_ih_b, b_hh_b)

    bias = np.asarray(bias, np.float32)
    brelu_m = np.zeros((128, 2), np.float32)
    brelu_t = np.zeros((22, 2), np.float32)
    for j in range(2):
        brelu_m[:, j] = bias[128 * j:128 * (j + 1)]
        brelu_t[:, j] = bias[256 + 22 * j:256 + 22 * (j + 1)]

    return {
        "wx_m_f": wx_m_f, "wx_t_f": wx_t_f, "wh_a_f": wh_a_f, "wh_b_f": wh_b_f,
        "wx_m_b": wx_m_b, "wx_t_b": wx_t_b, "wh_a_b": wh_a_b, "wh_b_b": wh_b_b,
        "brelu_m": brelu_m, "brelu_t": brelu_t,
    }


def _run(in_maps, trace=False, **kw):
    nc = build_program()
    return run_bass_kernel_spmd(nc, in_maps, list(range(NCORES)), trace=trace, **kw)


def kernel(x, batch, num_moles, max_len, bias, w_ih_f, w_hh_f, b_ih_f, b_hh_f,
           w_ih_b, w_hh_b, b_ih_b, b_hh_b):
    x = np.asarray(x, np.float32)
    batch = np.asarray(batch)
    assert int(num_moles) == B and int(max_len) == L
    assert x.shape == (B * L, H)
    expected_batch = np.repeat(np.arange(B, dtype=batch.dtype), L)
    assert np.array_equal(batch, expected_batch), \
        "kernel assumes uniform 48-length molecules"

    shared = _prep_shared_inputs(
        bias, w_ih_f, w_hh_f, b_ih_f, b_hh_f,
        w_ih_b, w_hh_b, b_ih_b, b_hh_b)

    in_maps = [dict(shared,
                    xT=np.ascontiguousarray(
                        x[c * NLOC:(c + 1) * NLOC].T).astype(NPBF))
               for c in range(NCORES)]
    res = _run(in_maps).results
    return np.concatenate([res[c]["y"] for c in range(NCORES)], axis=0)


# revision 60
# speedup vs baseline: 1.0162x; 1.0162x over previous
"""BatchGRU TRN2 kernel v3: fp8 DoubleRow x-side + bf16 h-side.

Host ships x pre-transposed ([300, NLOC] per core) and weights pre-packed:
x-side weights fp8e4m3 scaled x4096 in DoubleRow pair layout ([128,2,900] +
[32,2,900] tail with the xn-bias ones-row), h-side weights bf16 scaled x4096
([128,2,900] + [45,900] tail with the r/z/n bias row). Gate pre-activations
accumulate in PSUM at scale 4096 (one [128,3,512] tile per unit = r|z|xn|hn
bank-packed); sigmoid/tanh apply scale=1/4096. Recurrent state is bf16 in
k-chunk layout ([128,2,256] + [64,256] with ones row); h_new stays f32r in
natural layout (y DMA + h_old), PE-transposed and cast to bf16 on the
PSUM->SBUF copy. Elementwise work is spread ACT/DVE/Pool.
"""
import numpy as np
from contextlib import ExitStack

try:
    import jax as _jax
    _jax.config.update("jax_compilation_cache_dir", "/root/problem/jax_cache")
    _jax.config.update("jax_persistent_cache_min_compile_time_secs", 10.0)
    _jax.config.update("jax_persistent_cache_min_entry_size_bytes", 0)
except Exception:
    pass

import ml_dtypes
import concourse.bacc as bacc
import concourse.tile as tile
from concourse import mybir
from concourse.bass_utils import run_bass_kernel_spmd

f32 = mybir.dt.float32
f32r = mybir.dt.float32r
bf16 = mybir.dt.bfloat16
f8 = mybir.dt.float8e4
AF = mybir.ActivationFunctionType
ALU = mybir.AluOpType
DR = mybir.MatmulPerfMode.DoubleRow

NPF8 = ml_dtypes.float8_e4m3fn
NPBF = ml_dtypes.bfloat16

H = 300
G = 3 * H
L = 48
B = 2048
NCORES = 8
BLOC = B // NCORES           # 256 molecules per core
NLOC = BLOC * L              # 12288 tokens per core
NMC = 2                      # molecule chunks of 128
S = 4096.0                   # preact scale carried in the weights
INV_S = 1.0 / S
XONE = 128.0                 # ones-row value in fp8 x tiles
NCHUNK = 8                   # prologue token-column chunks
CW = NLOC // NCHUNK          # 3072 token cols per chunk
CM = BLOC // NCHUNK          # 64 molecules per chunk

# flat PSUM gate layout: r 0:300 | z 300:600 | xn 600:900 | hn 900:1200
# split at PSUM bank boundaries (512) and <=256-col pieces for DoubleRow.
# (flat0, flat1, has_x, has_h); weight col = flat (x) / flat-300*... see emit.
SUBREGIONS = [
    (0, 256, True, True),
    (256, 300, True, True),
    (300, 512, True, True),
    (512, 600, True, True),
    (600, 856, True, False),
    (856, 900, True, False),
    (900, 1024, False, True),
    (1024, 1200, False, True),
]

_cached = {}


def build_program():
    if "nc" in _cached:
        return _cached["nc"]
    nc = bacc.Bacc("TRN2", target_bir_lowering=False, debug=False,
                   dynamic_dma_scratch_size=512)

    xT_d = nc.declare_dram_parameter("xT", [H, NLOC], bf16, isOutput=False)
    wx_m_d = {d: nc.declare_dram_parameter(f"wx_m_{d}", [128, 2, G], f8,
                                           isOutput=False) for d in "fb"}
    wx_t_d = {d: nc.declare_dram_parameter(f"wx_t_{d}", [32, 2, G], f8,
                                           isOutput=False) for d in "fb"}
    wh_a_d = {d: nc.declare_dram_parameter(f"wh_a_{d}", [128, 2, G], bf16,
                                           isOutput=False) for d in "fb"}
    wh_b_d = {d: nc.declare_dram_parameter(f"wh_b_{d}", [45, G], bf16,
                                           isOutput=False) for d in "fb"}
    brelu_m_d = nc.declare_dram_parameter("brelu_m", [128, 2], f32, isOutput=False)
    brelu_t_d = nc.declare_dram_parameter("brelu_t", [22, 2], f32, isOutput=False)
    y_d = nc.declare_dram_parameter("y", [NLOC, 2 * H], bf16, isOutput=True)

    y_r = y_d[:].rearrange("(m l) c -> m l c", l=L)  # [256, 48, 600]
    # p-major view for single-DMA stores of the merged [128, 2, H] hnat tile
    y_p = y_d[:].rearrange("(mc p l) c -> p mc l c", mc=NMC, l=L)

    with tile.TileContext(nc) as tc:
        with ExitStack() as ctx:
            consts = ctx.enter_context(tc.tile_pool(name="consts", bufs=1))

            # fp8 relu(x+bias), DoubleRow pair layout; tail holds ones row at
            # partition 22 (j=0) for the xn bias.
            xq_m = consts.tile([128, 2, NLOC], f8, name="xq_m")
            xq_t = consts.tile([32, 2, NLOC], f8, name="xq_t")
            wx_m = {d: consts.tile([128, 2, G], f8, name=f"wx_m_{d}") for d in "fb"}
            wx_t = {d: consts.tile([32, 2, G], f8, name=f"wx_t_{d}") for d in "fb"}
            wh_a = {d: consts.tile([128, 2, G], bf16, name=f"wh_a_{d}") for d in "fb"}
            wh_b = {d: consts.tile([45, G], bf16, name=f"wh_b_{d}") for d in "fb"}
            brelu_m = consts.tile([128, 2], f32)
            brelu_t = consts.tile([22, 2], f32)
            # bf16 state, k-chunk layout: a: hdim 128j+p; b: hdim 256+p (p<44),
            # ones row at p=44 (memset covers 32:64).
            state_a = {d: consts.tile([128, 2, BLOC], bf16, name=f"sa_{d}")
                       for d in "fb"}
            state_b = {d: consts.tile([64, BLOC], bf16, name=f"sb_{d}")
                       for d in "fb"}
            hnat = {d: consts.tile([128, NMC, H], bf16, name=f"hnat_{d}")
                    for d in "fb"}
            id128r = consts.tile([128, 128], f32r)
            id_bf = consts.tile([128, 128], bf16)

            # ---- identities ----
            with tc.tile_pool(name="idp", bufs=1) as idp:
                rowi = idp.tile([128, 1], mybir.dt.int32)
                coli = idp.tile([128, 128], mybir.dt.int32)
                nc.gpsimd.iota(rowi, pattern=[[0, 1]], base=0, channel_multiplier=1)
                nc.gpsimd.iota(coli, pattern=[[1, 128]], base=0, channel_multiplier=0)
                rowf = idp.tile([128, 1], f32)
                colf = idp.tile([128, 128], f32)
                nc.vector.tensor_copy(out=rowf, in_=rowi)
                nc.vector.tensor_copy(out=colf, in_=coli)
                idf = idp.tile([128, 128], f32)
                nc.vector.tensor_scalar(out=idf, in0=colf, scalar1=rowf,
                                        scalar2=None, op0=ALU.is_equal)
                nc.scalar.activation(out=id128r, in_=idf, func=AF.Copy)
                nc.scalar.activation(out=id_bf, in_=idf, func=AF.Copy)

            # ---- weights + biases ----
            for d in "fb":
                nc.sync.dma_start(out=wx_m[d], in_=wx_m_d[d][:])
                nc.sync.dma_start(out=wx_t[d], in_=wx_t_d[d][:])
                nc.sync.dma_start(out=wh_a[d], in_=wh_a_d[d][:])
                nc.sync.dma_start(out=wh_b[d], in_=wh_b_d[d][:])
            nc.sync.dma_start(out=brelu_m, in_=brelu_m_d[:])
            nc.sync.dma_start(out=brelu_t, in_=brelu_t_d[:])

            # ones rows (overwritten on 0:22 by the relu writers below);
            # on Pool: these are 12288-wide and DVE is the prologue bottleneck
            nc.gpsimd.memset(xq_t[0:32, 0, :], XONE)
            nc.gpsimd.memset(xq_t[0:32, 1, :], 0.0)
            nc.gpsimd.memset(state_b["f"][32:64, :], 1.0)

            # ---- prologue: chunked x load, relu->fp8, h0 reduce ----
            with tc.tile_pool(name="xstage", bufs=2) as xstage:
                for c in range(NCHUNK):
                    cs = slice(c * CW, (c + 1) * CW)
                    ms = slice(c * CM, (c + 1) * CM)
                    st = xstage.tile([128, 2, CW], bf16, tag="stm")
                    stt = xstage.tile([22, 2, CW], bf16, tag="stt")
                    nc.sync.dma_start(out=st[:, 0, :], in_=xT_d[0:128, cs])
                    nc.sync.dma_start(out=st[:, 1, :], in_=xT_d[128:256, cs])
                    nc.sync.dma_start(out=stt[:, 0, :], in_=xT_d[256:278, cs])
                    nc.sync.dma_start(out=stt[:, 1, :], in_=xT_d[278:300, cs])
                    for j in range(2):
                        nc.scalar.activation(out=xq_m[:, j, cs], in_=st[:, j, :],
                                             func=AF.Relu,
                                             bias=brelu_m[:, j:j + 1], scale=1.0)
                        nc.scalar.activation(out=xq_t[0:22, j, cs], in_=stt[:, j, :],
                                             func=AF.Relu,
                                             bias=brelu_t[:, j:j + 1], scale=1.0)
                    # h0 = per-molecule max over raw x
                    st_v = st.rearrange("p j (m l) -> p j m l", l=L)
                    stt_v = stt.rearrange("p j (m l) -> p j m l", l=L)
                    for j in range(2):
                        nc.vector.tensor_reduce(out=state_a["f"][:, j, ms],
                                                in_=st_v[:, j],
                                                axis=mybir.AxisListType.X, op=ALU.max)
                    nc.vector.tensor_reduce(out=state_b["f"][0:22, ms],
                                            in_=stt_v[:, 0], axis=mybir.AxisListType.X,
                                            op=ALU.max)
                    # engine writes must start at a 32-aligned partition, so
                    # land the j=1 tail h0 at base 0 and DMA-shift it to 22:44
                    stgb = xstage.tile([22, CM], bf16, tag="stgb")  # noqa
                    nc.vector.tensor_reduce(out=stgb, in_=stt_v[:, 1],
                                            axis=mybir.AxisListType.X, op=ALU.max)
                    nc.sync.dma_start(out=state_b["f"][22:44, ms], in_=stgb)

            # state copy f -> b (incl. ones rows); ACT: DVE owns the reduces
            nc.scalar.activation(out=state_a["b"], in_=state_a["f"],
                                 func=AF.Copy)
            nc.scalar.activation(out=state_b["b"], in_=state_b["f"],
                                 func=AF.Copy)

            # h0 natural layout via PE transposes of the bf16 state
            with tc.tile_pool(name="h0t", bufs=2, space="PSUM") as h0t:
                for mc in range(NMC):
                    msl = slice(mc * 128, (mc + 1) * 128)
                    tp = h0t.tile([128, H], bf16, tag="h0")
                    nc.tensor.transpose(out=tp[:, 0:128],
                                        in_=state_a["f"][:, 0, msl], identity=id_bf)
                    nc.tensor.transpose(out=tp[:, 128:256],
                                        in_=state_a["f"][:, 1, msl], identity=id_bf)
                    nc.tensor.transpose(out=tp[:, 256:300],
                                        in_=state_b["f"][0:44, msl],
                                        identity=id_bf[0:44, 0:44])
                    nc.scalar.activation(out=hnat["f"][:, mc, :], in_=tp,
                                         func=AF.Copy)
                    nc.gpsimd.tensor_copy(out=hnat["b"][:, mc, :],
                                          in_=hnat["f"][:, mc, :])

            xqm_v = xq_m.rearrange("p j (m l) -> p j m l", l=L)
            xqt_v = xq_t.rearrange("p j (m l) -> p j m l", l=L)

            # ---- recurrence pools ----
            gp_pool = ctx.enter_context(tc.tile_pool(name="gp", bufs=2, space="PSUM"))
            tp_pool = ctx.enter_context(tc.tile_pool(name="tp", bufs=2, space="PSUM"))
            gates = ctx.enter_context(tc.tile_pool(name="gates", bufs=2))

            def emit_mm(d, s, mc):
                t = s if d == "f" else L - 1 - s
                sa, sb = state_a[d], state_b[d]
                msl = slice(mc * 128, (mc + 1) * 128)
                g_ps = gp_pool.tile([128, 3, 512], f32, tag="g",
                                    name=f"g_{d}{s}{mc}")
                gf = g_ps.rearrange("p a b -> p (a b)")
                for (f0, f1, has_x, has_h) in SUBREGIONS:
                    first = True
                    if has_x:
                        wsl = slice(f0, f1)
                        nc.tensor.matmul(
                            out=gf[:, f0:f1], lhsT=xqm_v[:, :, msl, t],
                            rhs=wx_m[d][:, :, wsl],
                            start=True, stop=False, perf_mode=DR)
                        nc.tensor.matmul(
                            out=gf[:, f0:f1], lhsT=xqt_v[0:23, :, msl, t],
                            rhs=wx_t[d][0:23, :, wsl],
                            start=False, stop=not has_h, perf_mode=DR)
                        first = False
                    if has_h:
                        w0 = f0 if f0 < 900 else f0 - 300
                        wsl = slice(w0, w0 + (f1 - f0))
                        for k in range(2):
                            nc.tensor.matmul(
                                out=gf[:, f0:f1], lhsT=sa[:, k, msl],
                                rhs=wh_a[d][:, k, wsl],
                                start=first and k == 0, stop=False)
                        nc.tensor.matmul(
                            out=gf[:, f0:f1], lhsT=sb[0:45, msl],
                            rhs=wh_b[d][0:45, wsl], start=False, stop=True)
                return gf

            def emit_gm(d, s, mc, gf):
                # gate math chain only; transposes/copies are emitted
                # separately (deferred for cross-step pipelining)
                t = s if d == "f" else L - 1 - s
                dcol = 0 if d == "f" else 1
                hn_ = hnat[d][:, mc, :]
                msl = slice(mc * 128, (mc + 1) * 128)
                rz = gates.tile([128, 2, H], f32, tag="rz", name=f"rz_{d}{s}{mc}")
                t1 = gates.tile([128, H], f32, tag="t1", name=f"t1_{d}{s}{mc}")
                n_sb = gates.tile([128, H], f32, tag="ns", name=f"ns_{d}{s}{mc}")
                nc.scalar.activation(out=rz, in_=gf[:, 0:600],
                                     func=AF.Sigmoid, scale=INV_S)
                nc.vector.tensor_mul(out=t1, in0=rz[:, 0, :], in1=gf[:, 900:1200])
                nc.vector.tensor_add(out=t1, in0=t1, in1=gf[:, 600:900])
                nc.scalar.activation(out=n_sb, in_=t1, func=AF.Tanh, scale=INV_S)
                nc.gpsimd.tensor_sub(out=t1, in0=hn_, in1=n_sb)
                nc.vector.tensor_mul(out=t1, in0=rz[:, 1, :], in1=t1)
                nc.gpsimd.tensor_add(out=hn_, in0=n_sb, in1=t1)
                if mc == NMC - 1:
                    # one DMA per (d, step): both mc chunks from the merged
                    # hnat tile; DRAM AP iterates (p, mc, c) to match
                    nc.sync.dma_start(
                        out=y_p[:, :, t, dcol * H:(dcol + 1) * H],
                        in_=hnat[d])

            def emit_tp(d, s, mc):
                hn_ = hnat[d][:, mc, :]
                tp = tp_pool.tile([128, 384], bf16, tag="tp", name=f"tp_{d}{s}{mc}")
                nc.tensor.transpose(out=tp[:, 0:128], in_=hn_[:, 0:128],
                                    identity=id_bf)
                nc.tensor.transpose(out=tp[:, 128:256], in_=hn_[:, 128:256],
                                    identity=id_bf)
                nc.tensor.transpose(out=tp[0:44, 256:384], in_=hn_[:, 256:300],
                                    identity=id_bf)
                return tp

            def emit_copies(d, mc, tp):
                sa, sb = state_a[d], state_b[d]
                msl = slice(mc * 128, (mc + 1) * 128)
                # split across ACT/DVE: the copies gate the next step's
                # h-side matmuls, so parallelize rather than batch
                nc.scalar.activation(out=sa[:, 0, msl],
                                     in_=tp[:, 0:128], func=AF.Copy)
                nc.vector.tensor_copy(out=sa[:, 1, msl],
                                      in_=tp[:, 128:256])
                nc.vector.tensor_copy(out=sb[0:44, msl],
                                      in_=tp[0:44, 256:384])

            for s in range(L):
                for d in "fb":
                    gfs = [emit_mm(d, s, mc) for mc in range(NMC)]
                    for mc in range(NMC):
                        emit_gm(d, s, mc, gfs[mc])
                        if s < L - 1:  # final state is never read back
                            tp = emit_tp(d, s, mc)
                            emit_copies(d, mc, tp)

    nc.compile()
    _cached["nc"] = nc
    return nc


def _prep_shared_inputs(bias, w_ih_f, w_hh_f, b_ih_f, b_hh_f,
                        w_ih_b, w_hh_b, b_ih_b, b_hh_b):
    def pack_dir(w_ih, w_hh, b_ih, b_hh):
        Wx = (np.asarray(w_ih, np.float64).T * S).astype(np.float32)  # [300, 900]
        Wh = (np.asarray(w_hh, np.float64).T * S).astype(np.float32)
        wx_m = np.zeros((128, 2, G), np.float32)
        wx_t = np.zeros((32, 2, G), np.float32)
        for j in range(2):
            wx_m[:, j, :] = Wx[128 * j:128 * (j + 1), :]
            wx_t[0:22, j, :] = Wx[256 + 22 * j:256 + 22 * (j + 1), :]
        # xn bias rides the fp8 ones row (value XONE) at partition 22, j=0
        wx_t[22, 0, 2 * H:] = np.asarray(b_ih, np.float64)[2 * H:] * (S / XONE)
        wh_a = np.zeros((128, 2, G), np.float32)
        for j in range(2):
            wh_a[:, j, :] = Wh[128 * j:128 * (j + 1), :]
        wh_b = np.zeros((45, G), np.float32)
        wh_b[0:44, :] = Wh[256:300, :]
        brow = np.asarray(b_hh, np.float64).copy()
        brow[:2 * H] += np.asarray(b_ih, np.float64)[:2 * H]
        wh_b[44, :] = (brow * S).astype(np.float32)
        return (wx_m.astype(NPF8), wx_t.astype(NPF8),
                wh_a.astype(NPBF), wh_b.astype(NPBF))

    wx_m_f, wx_t_f, wh_a_f, wh_b_f = pack_dir(w_ih_f, w_hh_f, b_ih_f, b_hh_f)
    wx_m_b, wx_t_b, wh_a_b, wh_b_b = pack_dir(w_ih_b, w_hh_b, b_ih_b, b_hh_b)

    bias = np.asarray(bias, np.float32)
    brelu_m = np.zeros((128, 2), np.float32)
    brelu_t = np.zeros((22, 2), np.float32)
    for j in range(2):
        brelu_m[:, j] = bias[128 * j:128 * (j + 1)]
        brelu_t[:, j] = bias[256 + 22 * j:256 + 22 * (j + 1)]

    return {
        "wx_m_f": wx_m_f, "wx_t_f": wx_t_f, "wh_a_f": wh_a_f, "wh_b_f": wh_b_f,
        "wx_m_b": wx_m_b, "wx_t_b": wx_t_b, "wh_a_b": wh_a_b, "wh_b_b": wh_b_b,
        "brelu_m": brelu_m, "brelu_t": brelu_t,
    }


def _run(in_maps, trace=False, **kw):
    nc = build_program()
    return run_bass_kernel_spmd(nc, in_maps, list(range(NCORES)), trace=trace, **kw)


def kernel(x, batch, num_moles, max_len, bias, w_ih_f, w_hh_f, b_ih_f, b_hh_f,
           w_ih_b, w_hh_b, b_ih_b, b_hh_b):
    x = np.asarray(x, np.float32)
    batch = np.asarray(batch)
    assert int(num_moles) == B and int(max_len) == L
    assert x.shape == (B * L, H)
    expected_batch = np.repeat(np.arange(B, dtype=batch.dtype), L)
    assert np.array_equal(batch, expected_batch), \
        "kernel assumes uniform 48-length molecules"

    shared = _prep_shared_inputs(
        bias, w_ih_f, w_hh_f, b_ih_f, b_hh_f,
        w_ih_b, w_hh_b, b_ih_b, b_hh_b)

    in_maps = [dict(shared,
                    xT=np.ascontiguousarray(
                        x[c * NLOC:(c + 1) * NLOC].T).astype(NPBF))
               for c in range(NCORES)]
    res = _run(in_maps).results
    return np.concatenate([np.asarray(res[c]["y"]) for c in range(NCORES)],
                          axis=0).astype(np.float32)
